# revision 72
# baseline (speedup 1.0000x reference)
"""MultiHeadLatentAttention TRN2 kernel — 8-core batch(2) x head-group(4)
parallel. ~390us HW exec vs the 534us head-sharded v1 baseline.

Design:
  - Sharding: core c handles batch c//4 and heads 4*(c%4)..4*(c%4)+3.
    Per-core x input halves (8.4MB bf16) and the out-projection partial
    halves ([2048, 2048]); host sums 4 bf16 partials per batch in f32.
  - Projections: q is weight-absorbed on host (W_q_down @ [W_qc|W_qr],
    1/sqrt(HD) folded in); k/v go through the kv latent on device
    (c_kv = x @ W_kv_down once, then K=512 up-projections per head) which
    is 16 fewer K=2048 chains per 512-token sub. The shared rope-key chain
    rides in head-0's kc PSUM rows 64:128; RoPE rotation via PE
    permutation matmul + DVE mul/add, applied once and copied to heads
    1-3. c_kv for sub q+1 is emitted at the end of sub q so its drains
    complete during attention quad q.
  - Attention: interleaved with projections (sub q feeds quad q, mixing
    the PE-heavy projection stream with the ACT-heavy softmax stream).
    Scores are computed TRANSPOSED (scoresT[k,q] = kT_blk^T @ qT) in
    quads of 4 q-blocks (N up to 512, exact causal trimming per 128-block
    + one 0/1-tri mask mult per diagonal block), exp'd on ACT straight
    into bf16 SBUF, and fed directly to the AV matmul
    (ctxT[hd,q] += vN_blk^T @ expT) — no attention transposes, no PSUM
    round-trips, no running-max (logits are bounded, |s| < ~4).
  - Softmax denominators ride side-chains: per-block exp-sums accumulate
    f32 on GpSimd (even blocks) + DVE (odd blocks); one all-ones [128,128]
    matmul then yields partition-sums replicated across all rows, and
    1/x is computed as Exp(-Ln(x)) on ACT (DVE Reciprocal is an 8
    cycle/elem iterative divide). Normalization folds into the ctx
    PSUM->SBUF copy. Each head's finalize chain and AV tail are deferred
    into the next head's section so no engine queue blocks another at
    head boundaries; the out-projection is deferred one quad and emitted
    one q-block per head section as an ACT catch-up window.
  - All operands host-packed into SBUF layouts (contiguous >=2KB DMA
    rows); DMA issue order gates the first matmul on ~1.5MB.
  - Env: MLA_DT=bf16|f32 stream dtype; MLA_OUT=bf16|f32 partial dtype.
"""
import functools
import os
import numpy as np

import concourse.bass as bass
import concourse.mybir as mybir
from concourse.tile import TileContext
from concourse.bass_utils import run_bass_kernel_spmd

F32 = mybir.dt.float32
AF = mybir.ActivationFunctionType
ALU = mybir.AluOpType

NC = 8           # cores
NB = 2           # batch shards
NG = 4           # head-group shards
HL = 4           # heads per core
B, S, D = 2, 2048, 2048
H, HD, RD, CD = 16, 128, 64, 64
ND = D // 128    # 16 contraction chunks
SUB = 512        # projection sub-chunk (tokens, = PSUM N = xt stage)
NSUB = S // SUB  # 4
QB = 128
NQB = S // QB    # 16
NQUAD = NQB // 4  # 4 quads of 4 q-blocks

KVL = 512        # kv latent dim
_DT_NAME = os.environ.get("MLA_DT", "bf16")     # bf16 | f32
_OUT_NAME = os.environ.get("MLA_OUT", "bf16")   # bf16 | f32

_CACHE = {}


# ---------------------------------------------------------------------------
# Tile-on-this-walrus compat: max ONE sync wait per instruction. Extra waits
# are hoisted onto wait-only EventSemaphore instructions inserted just before
# the over-subscribed instruction on the same engine (program order makes
# this equivalent).
# ---------------------------------------------------------------------------
def _split_multi_waits(nc, max_waits=1):
    n = 0
    for f in nc.m.functions:
        for bb in f.blocks:
            new_insts = []
            for ins in bb.instructions:
                si = ins.sync_info
                waits = list(si.on_wait) if si is not None else []
                if len(waits) > max_waits:
                    extra, keep = waits[:-max_waits], waits[-max_waits:]
                    for j, w in enumerate(extra):
                        ev = mybir.InstEventSemaphore(
                            name=f"{ins.name}_xw{j}",
                            engine=ins.engine,
                            ins=[],
                            outs=[],
                            sync_info=mybir.SyncInfo(on_wait=[w], on_update=[]),
                        )
                        new_insts.append(ev)
                        n += 1
                    ins.sync_info = mybir.SyncInfo(
                        on_wait=keep, on_update=list(si.on_update)
                    )
                new_insts.append(ins)
            bb.instructions[:] = new_insts
    return n


def _stream_dt():
    return mybir.dt.bfloat16 if _DT_NAME == "bf16" else mybir.dt.float32


def _out_dt():
    return mybir.dt.bfloat16 if _OUT_NAME == "bf16" else mybir.dt.float32


# ---------------------------------------------------------------------------
# Device program (SPMD: identical on all 8 cores, inputs differ per core)
# ---------------------------------------------------------------------------
def _build_program():
    DT = _stream_dt()
    ODT = _out_dt()
    nc = bass.Bass()

    xtp = nc.dram_tensor("xtp", [128, ND, S], DT, kind="ExternalInput")
    wqp = nc.dram_tensor("wqp", [128, ND, HL * HD], DT, kind="ExternalInput")
    wkvdp = nc.dram_tensor("wkvdp", [128, ND, KVL], DT, kind="ExternalInput")
    wkcp = nc.dram_tensor("wkcp", [128, KVL // 128, HL * CD], DT,
                          kind="ExternalInput")
    wkrp = nc.dram_tensor("wkrp", [128, ND, RD], DT, kind="ExternalInput")
    wvup = nc.dram_tensor("wvup", [128, KVL // 128, HL * HD], DT,
                          kind="ExternalInput")
    wop = nc.dram_tensor("wop", [128, HL, D], DT, kind="ExternalInput")
    cosd = nc.dram_tensor("cosd", [RD, S], DT, kind="ExternalInput")
    sind = nc.dram_tensor("sind", [RD, S], DT, kind="ExternalInput")
    identd = nc.dram_tensor("identd", [128, 128], DT, kind="ExternalInput")
    p64d = nc.dram_tensor("p64d", [RD, RD], DT, kind="ExternalInput")
    trid = nc.dram_tensor("trid", [128, 128], DT, kind="ExternalInput")
    onesd = nc.dram_tensor("onesd", [128, 128], DT, kind="ExternalInput")
    outd = nc.dram_tensor("out", [S, D], ODT, kind="ExternalOutput")

    with TileContext(nc) as tc:
        with tc.tile_pool(name="stat", bufs=1) as stat, \
             tc.tile_pool(name="seq", bufs=1) as seq, \
             tc.tile_pool(name="xtc", bufs=2) as xtc, \
             tc.tile_pool(name="atp", bufs=9) as atp, \
             tc.tile_pool(name="ctxp", bufs=2) as ctxp, \
             tc.tile_pool(name="stream", bufs=4) as stream, \
             tc.tile_pool(name="small", bufs=2) as small, \
             tc.tile_pool(name="psA", bufs=2, space="PSUM") as psA, \
             tc.tile_pool(name="scr", bufs=3, space="PSUM") as scr, \
             tc.tile_pool(name="accp", bufs=2, space="PSUM") as accp:

            # ---- staged constants/weights; DMA order = priority order ----
            # (first projection chain is gated only on xt[d0:4] + wq[d0:4])
            def stage_xt(sub):
                xt = xtc.tile([128, ND, SUB], DT, tag="xt", name=f"xt{sub}")
                csl = slice(sub * SUB, (sub + 1) * SUB)
                for dpart in range(0, ND, 4):
                    dsl = slice(dpart, dpart + 4)
                    nc.sync.dma_start(out=xt[:, dsl, :], in_=xtp[:, dsl, csl])
                return xt

            wq_sb = stat.tile([128, ND, HL * HD], DT, tag="wq")
            wkvd_sb = stat.tile([128, ND, KVL], DT, tag="wkvd")
            xt0 = xtc.tile([128, ND, SUB], DT, tag="xt", name="xt0")
            for dpart in range(0, ND, 4):
                dsl = slice(dpart, dpart + 4)
                nc.sync.dma_start(out=xt0[:, dsl, :], in_=xtp[:, dsl, 0:SUB])
                nc.sync.dma_start(out=wkvd_sb[:, dsl, :],
                                  in_=wkvdp[:, dsl, :])
                nc.sync.dma_start(out=wq_sb[:, dsl, :], in_=wqp[:, dsl, :])
            wkc_sb = stat.tile([128, KVL // 128, HL * CD], DT, tag="wkc")
            nc.sync.dma_start(out=wkc_sb[:], in_=wkcp[:])
            wvu_sb = stat.tile([128, KVL // 128, HL * HD], DT, tag="wvu")
            nc.sync.dma_start(out=wvu_sb[:], in_=wvup[:])
            wkr_sb = stat.tile([128, ND, RD], DT, tag="wkr")
            nc.sync.dma_start(out=wkr_sb[:], in_=wkrp[:])
            cosT = stat.tile([128, S], DT, tag="cos")
            sinT = stat.tile([128, S], DT, tag="sin")
            nc.sync.dma_start(out=cosT[64:128, :], in_=cosd[:])
            nc.sync.dma_start(out=sinT[64:128, :], in_=sind[:])
            ident = stat.tile([128, 128], DT, tag="ident")
            nc.sync.dma_start(out=ident[:], in_=identd[:])
            p64 = stat.tile([128, RD], DT, tag="p64")
            nc.sync.dma_start(out=p64[64:128, :], in_=p64d[:])
            tri = stat.tile([128, 128], DT, tag="tri")
            nc.sync.dma_start(out=tri[:], in_=trid[:])
            ones = stat.tile([128, 128], DT, tag="ones")
            nc.sync.dma_start(out=ones[:], in_=onesd[:])
            wo_sb = stat.tile([128, HL, D], DT, tag="wo")

            qT = [seq.tile([128, S], DT, tag=f"qT{l}", name=f"qT{l}")
                  for l in range(HL)]
            kT = [seq.tile([128, S], DT, tag=f"kT{l}", name=f"kT{l}")
                  for l in range(HL)]
            vN = [seq.tile([128, S], DT, tag=f"vN{l}", name=f"vN{l}")
                  for l in range(HL)]

            # ---------------- projections ----------------
            # chain order per sub: q(0..3), k(0..3), v(0..3) — the first 4
            # chains are gated only on wq+xt (4MB); wk/wv stream in their
            # shadow. RoPE rotations ride along the k chains; v transposes
            # are deferred one chain so the vt drain never stalls PE.
            def chain(w_sb, l, xt):
                ps = psA.tile([128, SUB], F32, tag="mm", name="ps")
                for d in range(ND):
                    nc.tensor.matmul(ps[:], w_sb[:, d, l * 128:(l + 1) * 128],
                                     xt[:, d, :], start=d == 0,
                                     stop=d == ND - 1)
                return ps

            def emit_vtp(vt_sb, l, sub):
                # vT chunk -> natural layout via PE transpose
                for s4 in range(0, SUB // 128, 2):
                    tp = scr.tile([128, 256], DT, tag="scr", name="tp")
                    nc.tensor.transpose(
                        tp[:, 0:128],
                        vt_sb[:, s4 * 128:(s4 + 1) * 128], ident[:])
                    nc.tensor.transpose(
                        tp[:, 128:256],
                        vt_sb[:, (s4 + 1) * 128:(s4 + 2) * 128], ident[:])
                    col = sub * SUB + s4 * 128
                    if s4 == 0:
                        nc.scalar.copy(vN[l][:, col:col + 256], tp[:, 0:256])
                    else:
                        nc.vector.tensor_copy(vN[l][:, col:col + 256],
                                              tp[:, 0:256])

            # kv-latent: c_kv = x @ W_kv_down once (4 chains), then K=512
            # up-projections per head — 16 fewer K=2048 chains per sub.
            # c_kv for sub q+1 is emitted at the END of sub q so its
            # PSUM->SBUF drains complete during attention quad q instead
            # of stalling the next sub's kc/v chains.
            def emit_ckv(xt):
                ckv = []
                for j in range(KVL // 128):
                    c_ps = chain(wkvd_sb, j, xt)
                    ck = small.tile([128, SUB], DT, tag=f"ckv{j}",
                                    name=f"ckv{j}")
                    if j % 2 == 0:
                        nc.scalar.copy(ck[:], c_ps[:])
                    else:
                        nc.vector.tensor_copy(ck[:], c_ps[:])
                    ckv.append(ck)
                return ckv

            def emit_proj_sub(sub, xt, ckv):
                sl = slice(sub * SUB, (sub + 1) * SUB)
                for l in range(HL):
                    q_ps = chain(wq_sb, l, xt)
                    nc.vector.tensor_copy(qT[l][:, sl], q_ps[:])
                vts = []
                for l in range(HL):
                    k_ps = psA.tile([128, SUB], F32, tag="mm", name="k_ps")
                    for j in range(KVL // 128):
                        nc.tensor.matmul(
                            k_ps[0:CD, :],
                            wkc_sb[:, j, l * CD:(l + 1) * CD],
                            ckv[j][:], start=j == 0, stop=j == KVL // 128 - 1)
                    if l == 0:
                        # shared rope key rides in head-0's psum rows 64:128
                        for d in range(ND):
                            nc.tensor.matmul(
                                k_ps[64:128, :], wkr_sb[:, d, :],
                                xt[:, d, :], start=d == 0, stop=d == ND - 1)
                    # v up-projection rides with kc: together they give PE
                    # enough work per head to cover the DVE rope chain
                    v_ps = psA.tile([128, SUB], F32, tag="mm", name="v_ps")
                    for j in range(KVL // 128):
                        nc.tensor.matmul(
                            v_ps[:], wvu_sb[:, j, l * HD:(l + 1) * HD],
                            ckv[j][:], start=j == 0, stop=j == KVL // 128 - 1)
                    if l == 0:
                        nc.vector.tensor_copy(kT[0][:, sl], k_ps[:])
                    else:
                        nc.vector.tensor_copy(kT[l][0:CD, sl], k_ps[0:CD, :])
                    vt_sb = small.tile([128, SUB], DT, tag="vtsb", bufs=3)
                    nc.scalar.copy(vt_sb[:], v_ps[:])
                    vts.append(vt_sb)
                    # rope rotation matmuls ride between chains
                    rot = scr.tile([128, SUB], F32, tag="scr")
                    nc.tensor.matmul(rot[0:64, :], p64[64:128, :],
                                     qT[l][64:128, sl], start=True, stop=True)
                    tmp = small.tile([128, SUB], F32, tag="ropetmp")
                    nc.vector.tensor_tensor(qT[l][64:128, sl],
                                            qT[l][64:128, sl],
                                            cosT[64:128, sl], op=ALU.mult)
                    nc.vector.tensor_tensor(tmp[64:128, :], rot[0:64, :],
                                            sinT[64:128, sl], op=ALU.mult)
                    nc.vector.tensor_tensor(qT[l][64:128, sl],
                                            qT[l][64:128, sl],
                                            tmp[64:128, :], op=ALU.add)
                    if l == 0:
                        rotk = scr.tile([128, SUB], F32, tag="scr")
                        nc.tensor.matmul(rotk[0:64, :], p64[64:128, :],
                                         kT[0][64:128, sl],
                                         start=True, stop=True)
                        tmpk = small.tile([128, SUB], F32, tag="ropetmp")
                        nc.vector.tensor_tensor(kT[0][64:128, sl],
                                                kT[0][64:128, sl],
                                                cosT[64:128, sl], op=ALU.mult)
                        nc.vector.tensor_tensor(tmpk[64:128, :], rotk[0:64, :],
                                                sinT[64:128, sl], op=ALU.mult)
                        nc.vector.tensor_tensor(kT[0][64:128, sl],
                                                kT[0][64:128, sl],
                                                tmpk[64:128, :], op=ALU.add)
                    else:
                        # shared rope key: reuse head 0's rotated rows
                        nc.vector.tensor_copy(kT[l][64:128, sl],
                                              kT[0][64:128, sl])
                    if l > 0:
                        emit_vtp(vts[l - 1], l - 1, sub)
                emit_vtp(vts[HL - 1], HL - 1, sub)

            # -------- attention (quads of 4 q-blocks) + out-projection -----
            LOOK = 3
            ctx_hist = {}

            def outproj_qh(q, qh):
                row0 = (4 * q + qh) * 128
                for n in range(4):
                    op_ps = psA.tile([128, 512], F32, tag="mm")
                    for l in range(HL):
                        nc.tensor.matmul(
                            op_ps[:],
                            ctx_hist[q][l][:, qh * 128:(qh + 1) * 128],
                            wo_sb[:, l, n * 512:(n + 1) * 512],
                            start=l == 0, stop=l == HL - 1)
                    ob = stream.tile([128, 512], ODT, tag="ob")
                    if n % 2 == 0:
                        nc.scalar.copy(ob[:], op_ps[:])
                    else:
                        nc.vector.tensor_copy(ob[:], op_ps[:])
                    nc.sync.dma_start(
                        out=outd[row0:row0 + 128, n * 512:(n + 1) * 512],
                        in_=ob[:])

            # finalize (softmax denominator + normalized ctx copy) for head
            # (q, l) is DEFERRED into the next head's section, split in two:
            # part A (DVE merge + PE rowsum matmul) lands early where its
            # inputs are long ready; part B (ACT Ln/Exp + DVE mult) lands
            # after the section's scores so it sits in the ACT idle slot
            # instead of delaying the next head's Exp stream.
            pend_a = [None]
            pend_b = [None]
            pend_tail = [None]      # deferred AV tail of the previous head

            def fin_a():
                st = pend_a[0]
                if st is None:
                    return
                pend_a[0] = None
                fq, fl, acc, racg, racv = st
                rbf = small.tile([128, 512], DT, tag="rbf")
                nc.vector.tensor_tensor(rbf[:], racg[:], racv[:], op=ALU.add)
                rs = accp.tile([128, 512], F32, tag="rs", bufs=1, name="rs")
                nc.tensor.matmul(rs[:], ones[:], rbf[:], start=True,
                                 stop=True)
                pend_b[0] = (fq, fl, acc, rs)

            def fin_b():
                st = pend_b[0]
                if st is None:
                    return
                pend_b[0] = None
                fq, fl, acc, rs = st
                lnr = small.tile([128, 512], F32, tag="lnr")
                nc.scalar.activation(lnr[:], rs[:], AF.Ln)
                rc = small.tile([128, 512], F32, tag="rc")
                nc.scalar.activation(rc[:], lnr[:], AF.Exp, scale=-1.0)
                ct = ctxp.tile([128, 512], DT, tag=f"ct{fl}")
                nc.vector.tensor_tensor(ct[:], acc[:], rc[:], op=ALU.mult)
                ctx_hist[fq].append(ct)

            def emit_attn_quad(q):
                q0 = 4 * q                      # first q-block of the quad
                nbk = 4 * q + 4                 # k blocks 0..nbk-1
                ctx_hist[q] = []
                for l in range(HL):
                    acc = accp.tile([128, 512], F32, tag="acc")
                    # exp-sum side chains, split even/odd blocks across the
                    # GpSimd and Vector engines (GpSimd alone can't keep pace)
                    racg = small.tile([128, 512], F32, tag="racg")
                    racv = small.tile([128, 512], F32, tag="racv",
                                      name="racv")
                    at_tiles = [None] * nbk

                    def emit_score(kb):
                        i0 = max(kb - q0, 0)    # first valid q-block index
                        W = 512 - 128 * i0
                        c0 = 512 - W
                        sT = scr.tile([128, 512], F32, tag="scr")
                        nc.tensor.matmul(
                            sT[:, 0:W],
                            kT[l][:, kb * 128:(kb + 1) * 128],
                            qT[l][:, q0 * 128 + c0: (q0 + 4) * 128],
                            start=True, stop=True)
                        at = atp.tile([128, 512], DT, tag="at")
                        nc.scalar.activation(at[:, 0:W], sT[:, 0:W], AF.Exp)
                        if kb >= q0:
                            # diagonal q-block sits at tile cols 0:128
                            # (on GpSimd: DVE is the attention-phase
                            # pacer, GpSimd has headroom)
                            nc.gpsimd.tensor_tensor(
                                at[:, 0:128], at[:, 0:128], tri[:],
                                op=ALU.mult)
                        eng, r = (nc.gpsimd, racg) if kb % 2 == 0 \
                            else (nc.vector, racv)
                        if kb < 2:
                            # memset + add beats tensor_copy: the bf16->f32
                            # cast copy runs ~1.9us vs memset 0.14 + add 0.7
                            eng.memset(r[:], 0.0)
                        eng.tensor_tensor(r[:, c0:512], r[:, c0:512],
                                          at[:, 0:W], op=ALU.add)
                        at_tiles[kb] = at

                    def emit_av(kb, acc=acc, at_tiles=at_tiles, l=l, q0=q0):
                        i0 = max(kb - q0, 0)
                        W = 512 - 128 * i0
                        c0 = 512 - W
                        at = at_tiles[kb]
                        vblk = vN[l][:, kb * 128:(kb + 1) * 128]
                        if kb < q0:
                            nc.tensor.matmul(acc[:, 0:512], vblk, at[:, 0:512],
                                             start=kb == 0, stop=False)
                        else:
                            # diag block: cols 0:128 of the tile finish
                            # q-block i0; the rest continue accumulating
                            nc.tensor.matmul(
                                acc[:, c0:c0 + 128], vblk, at[:, 0:128],
                                start=kb == 0, stop=True)
                            if W > 128:
                                nc.tensor.matmul(
                                    acc[:, c0 + 128:512], vblk, at[:, 128:W],
                                    start=kb == 0, stop=i0 == 3)

                    tail = pend_tail[0]
                    pend_tail[0] = None
                    for kb in range(nbk):
                        emit_score(kb)
                        # the previous head's AV tail rides in the first
                        # score slots, where its exps are long done
                        if tail and kb < len(tail):
                            tail[kb]()
                        if kb == 1:
                            fin_a()
                        if kb >= LOOK:
                            emit_av(kb - LOOK)
                    pend_tail[0] = [functools.partial(emit_av, kb)
                                    for kb in range(nbk - LOOK, nbk)]

                    fin_b()
                    pend_a[0] = (q, l, acc, racg, racv)
                    # interleave one q-block of the previous quad's
                    # out-projection per head: PE burst with no ACT deps,
                    # so the Exp pipeline gets a catch-up window
                    if q > 0:
                        outproj_qh(q - 1, l)

            # ---- driver: projection sub q feeds attention quad q; the
            # interleave mixes the PE-heavy/ACT-light projection stream
            # with the ACT-heavy attention stream so Exp never falls
            # behind, and spreads DMA in/out across the whole kernel ----
            xt_cur = xt0
            ckv_next = None
            for q in range(NQUAD):
                xt = xt_cur
                if q + 1 < NSUB:
                    xt_cur = stage_xt(q + 1)
                if q == 1:
                    # wo is first needed at the out-projection (quad 1);
                    # load it in the shadow of sub-1 projection compute
                    for lh in range(0, HL, 2):
                        nc.sync.dma_start(out=wo_sb[:, lh:lh + 2, :],
                                          in_=wop[:, lh:lh + 2, :])
                ckv_q = emit_ckv(xt) if q == 0 else ckv_next
                emit_proj_sub(q, xt, ckv_q)
                if q + 1 < NQUAD:
                    ckv_next = emit_ckv(xt_cur)
                emit_attn_quad(q)
            for f in pend_tail[0] or []:
                f()
            fin_a()
            fin_b()
            for qh in range(4):
                outproj_qh(NQUAD - 1, qh)

    return nc


# ---------------------------------------------------------------------------
# Host side
# ---------------------------------------------------------------------------
def _rope_tables():
    inv_freq = 1.0 / (10000.0 ** (np.arange(0, RD, 2, dtype=np.float32) / RD))
    t = np.arange(S, dtype=np.float32)
    freqs = np.outer(t, inv_freq).astype(np.float32)
    emb = np.concatenate([freqs, freqs], axis=-1)
    cos = np.cos(emb).astype(np.float32)    # [S, RD]
    sin = np.sin(emb).astype(np.float32)
    return np.ascontiguousarray(cos.T), np.ascontiguousarray(sin.T)


def _host_prep(x, W_kv_down, W_q_down, W_kc, W_v, W_qc, W_kr, W_qr, W_o, b_o):
    f = np.float32
    Wqc_f = (W_q_down @ W_qc).astype(f)       # [D, CD*H]
    Wqr_f = (W_q_down @ W_qr).astype(f)       # [D, RD*H]
    scale = f(1.0 / np.sqrt(np.float32(HD)))

    cosT, sinT = _rope_tables()

    ident = np.eye(128, dtype=f)
    p64 = np.zeros((RD, RD), f)
    for m in range(RD):
        if m < 32:
            p64[m + 32, m] = -1.0
        else:
            p64[m - 32, m] = 1.0
    tri01 = (np.arange(128)[:, None] <= np.arange(128)[None, :]).astype(f)
    ones128 = np.ones((128, 128), f)

    # packed per-group weights
    wq_g, wkc_g, wvu_g, wo_g = [], [], [], []
    for g in range(NG):
        wq_c = np.empty((D, HL * HD), f)
        wkc_c = np.empty((KVL, HL * CD), f)
        wvu_c = np.empty((KVL, HL * HD), f)
        wo_c = np.empty((HL * HD, D), f)
        for l in range(HL):
            h = HL * g + l
            wq_c[:, l * 128:l * 128 + 64] = \
                Wqc_f[:, h * 64:(h + 1) * 64] * scale
            wq_c[:, l * 128 + 64:(l + 1) * 128] = \
                Wqr_f[:, h * 64:(h + 1) * 64] * scale
            wkc_c[:, l * CD:(l + 1) * CD] = W_kc[:, h * CD:(h + 1) * CD]
            wvu_c[:, l * HD:(l + 1) * HD] = W_v[:, h * HD:(h + 1) * HD]
            wo_c[l * 128:(l + 1) * 128, :] = W_o[h * 128:(h + 1) * 128, :]
        # [D, C] -> [128, ND, C];  [KVL, C] -> [128, KVL//128, C]
        wq_g.append(np.ascontiguousarray(
            wq_c.reshape(ND, 128, HL * HD).transpose(1, 0, 2)))
        wkc_g.append(np.ascontiguousarray(
            wkc_c.reshape(KVL // 128, 128, HL * CD).transpose(1, 0, 2)))
        wvu_g.append(np.ascontiguousarray(
            wvu_c.reshape(KVL // 128, 128, HL * HD).transpose(1, 0, 2)))
        # [HL*128, D] -> [128, HL, D]
        wo_g.append(np.ascontiguousarray(
            wo_c.reshape(HL, 128, D).transpose(1, 0, 2)))
    wkvd_p = np.ascontiguousarray(
        W_kv_down.astype(f).reshape(ND, 128, KVL).transpose(1, 0, 2))
    wkr_p = np.ascontiguousarray(
        W_kr.astype(f).reshape(ND, 128, RD).transpose(1, 0, 2))

    # packed x per batch: [128, ND, S];  xtp[p, d, t] = x[b, t, d*128+p]
    xtp_b = []
    for b in range(NB):
        xb = x[b]                              # [S, D]
        xtp_b.append(np.ascontiguousarray(
            xb.T.reshape(ND, 128, S).transpose(1, 0, 2)))

    in_maps = []
    for c in range(NC):
        b, g = c // NG, c % NG
        in_maps.append({
            "xtp": xtp_b[b], "wqp": wq_g[g], "wkvdp": wkvd_p,
            "wkcp": wkc_g[g], "wkrp": wkr_p, "wvup": wvu_g[g],
            "wop": wo_g[g],
            "cosd": cosT, "sind": sinT,
            "identd": ident, "p64d": p64, "trid": tri01, "onesd": ones128,
        })
    if _DT_NAME == "bf16":
        import ml_dtypes
        bf = ml_dtypes.bfloat16
        in_maps = [{k: v.astype(bf) for k, v in m.items()} for m in in_maps]
    return in_maps


def kernel(**inputs):
    inputs = {k: np.asarray(v, np.float32) for k, v in inputs.items()}
    if "nc" not in _CACHE:
        prog = _build_program()
        _split_multi_waits(prog)
        _CACHE["nc"] = prog
    prog = _CACHE["nc"]
    in_maps = _host_prep(**inputs)
    res = None
    for attempt in range(3):
        try:
            res = run_bass_kernel_spmd(prog, in_maps, core_ids=list(range(NC)))
            break
        except Exception:
            if attempt == 2:
                raise
            import time
            time.sleep(5.0)
    out = np.zeros((B, S, D), np.float32)
    for c, r in enumerate(res.results):
        out[c // NG] += np.asarray(r["out"], np.float32)
    out += inputs["b_o"][None, None, :]
    return out


# revision 75
# speedup vs baseline: 1.0498x; 1.0498x over previous
"""MultiHeadLatentAttention TRN2 kernel — 8-core batch(2) x head-group(4)
parallel. ~390us HW exec vs the 534us head-sharded v1 baseline.

Design:
  - Sharding: core c handles batch c//4 and heads 4*(c%4)..4*(c%4)+3.
    Per-core x input halves (8.4MB bf16) and the out-projection partial
    halves ([2048, 2048]); host sums 4 bf16 partials per batch in f32.
  - Projections: q is weight-absorbed on host (W_q_down @ [W_qc|W_qr],
    1/sqrt(HD) folded in); k/v go through the kv latent on device
    (c_kv = x @ W_kv_down once, then K=512 up-projections per head) which
    is 16 fewer K=2048 chains per 512-token sub. The shared rope-key chain
    rides in head-0's kc PSUM rows 64:128; RoPE rotation via PE
    permutation matmul + DVE mul/add, applied once and copied to heads
    1-3. c_kv for sub q+1 is emitted at the end of sub q so its drains
    complete during attention quad q.
  - Attention: interleaved with projections (sub q feeds quad q, mixing
    the PE-heavy projection stream with the ACT-heavy softmax stream).
    Scores are computed TRANSPOSED (scoresT[k,q] = kT_blk^T @ qT) in
    quads of 4 q-blocks (N up to 512, exact causal trimming per 128-block
    + one 0/1-tri mask mult per diagonal block), exp'd on ACT straight
    into bf16 SBUF, and fed directly to the AV matmul
    (ctxT[hd,q] += vN_blk^T @ expT) — no attention transposes, no PSUM
    round-trips, no running-max (logits are bounded, |s| < ~4).
  - Softmax denominators ride side-chains: per-block exp-sums accumulate
    f32 on GpSimd (even blocks) + DVE (odd blocks); one all-ones [128,128]
    matmul then yields partition-sums replicated across all rows, and
    1/x is computed as Exp(-Ln(x)) on ACT (DVE Reciprocal is an 8
    cycle/elem iterative divide). Normalization folds into the ctx
    PSUM->SBUF copy. Each head's finalize chain and AV tail are deferred
    into the next head's section so no engine queue blocks another at
    head boundaries; the out-projection is deferred one quad and emitted
    one q-block per head section as an ACT catch-up window.
  - All operands host-packed into SBUF layouts (contiguous >=2KB DMA
    rows); DMA issue order gates the first matmul on ~1.5MB.
  - Env: MLA_DT=bf16|f32 stream dtype; MLA_OUT=bf16|f32 partial dtype.
"""
import functools
import os
import numpy as np

import concourse.bass as bass
import concourse.mybir as mybir
from concourse.tile import TileContext
from concourse.bass_utils import run_bass_kernel_spmd

F32 = mybir.dt.float32
AF = mybir.ActivationFunctionType
ALU = mybir.AluOpType

NC = 8           # cores
NB = 2           # batch shards
NG = 4           # head-group shards
HL = 4           # heads per core
B, S, D = 2, 2048, 2048
H, HD, RD, CD = 16, 128, 64, 64
ND = D // 128    # 16 contraction chunks
SUB = 512        # projection sub-chunk (tokens, = PSUM N = xt stage)
NSUB = S // SUB  # 4
QB = 128
NQB = S // QB    # 16
NQUAD = NQB // 4  # 4 quads of 4 q-blocks

KVL = 512        # kv latent dim
_DT_NAME = os.environ.get("MLA_DT", "bf16")     # bf16 | f32
_OUT_NAME = os.environ.get("MLA_OUT", "bf16")   # bf16 | f32

_CACHE = {}


# ---------------------------------------------------------------------------
# Tile-on-this-walrus compat: max ONE sync wait per instruction. Extra waits
# are hoisted onto wait-only EventSemaphore instructions inserted just before
# the over-subscribed instruction on the same engine (program order makes
# this equivalent).
# ---------------------------------------------------------------------------
def _split_multi_waits(nc, max_waits=1):
    n = 0
    for f in nc.m.functions:
        for bb in f.blocks:
            new_insts = []
            for ins in bb.instructions:
                si = ins.sync_info
                waits = list(si.on_wait) if si is not None else []
                if len(waits) > max_waits:
                    extra, keep = waits[:-max_waits], waits[-max_waits:]
                    for j, w in enumerate(extra):
                        ev = mybir.InstEventSemaphore(
                            name=f"{ins.name}_xw{j}",
                            engine=ins.engine,
                            ins=[],
                            outs=[],
                            sync_info=mybir.SyncInfo(on_wait=[w], on_update=[]),
                        )
                        new_insts.append(ev)
                        n += 1
                    ins.sync_info = mybir.SyncInfo(
                        on_wait=keep, on_update=list(si.on_update)
                    )
                new_insts.append(ins)
            bb.instructions[:] = new_insts
    return n


def _stream_dt():
    return mybir.dt.bfloat16 if _DT_NAME == "bf16" else mybir.dt.float32


def _out_dt():
    return mybir.dt.bfloat16 if _OUT_NAME == "bf16" else mybir.dt.float32


# ---------------------------------------------------------------------------
# Device program (SPMD: identical on all 8 cores, inputs differ per core)
# ---------------------------------------------------------------------------
def _build_program():
    DT = _stream_dt()
    ODT = _out_dt()
    nc = bass.Bass()

    xtp = nc.dram_tensor("xtp", [128, ND, S], DT, kind="ExternalInput")
    wqp = nc.dram_tensor("wqp", [128, ND, HL * HD], DT, kind="ExternalInput")
    wkvdp = nc.dram_tensor("wkvdp", [128, ND, KVL], DT, kind="ExternalInput")
    wkcp = nc.dram_tensor("wkcp", [128, KVL // 128, HL * CD], DT,
                          kind="ExternalInput")
    wkrp = nc.dram_tensor("wkrp", [128, ND, RD], DT, kind="ExternalInput")
    wvup = nc.dram_tensor("wvup", [128, KVL // 128, HL * HD], DT,
                          kind="ExternalInput")
    wop = nc.dram_tensor("wop", [128, HL, D], DT, kind="ExternalInput")
    cosd = nc.dram_tensor("cosd", [RD, S], DT, kind="ExternalInput")
    sind = nc.dram_tensor("sind", [RD, S], DT, kind="ExternalInput")
    identd = nc.dram_tensor("identd", [128, 128], DT, kind="ExternalInput")
    p64d = nc.dram_tensor("p64d", [RD, RD], DT, kind="ExternalInput")
    trid = nc.dram_tensor("trid", [128, 128], DT, kind="ExternalInput")
    onesd = nc.dram_tensor("onesd", [128, 128], DT, kind="ExternalInput")
    outd = nc.dram_tensor("out", [S, D], ODT, kind="ExternalOutput")

    with TileContext(nc) as tc:
        with tc.tile_pool(name="stat", bufs=1) as stat, \
             tc.tile_pool(name="seq", bufs=1) as seq, \
             tc.tile_pool(name="xtc", bufs=2) as xtc, \
             tc.tile_pool(name="atp", bufs=9) as atp, \
             tc.tile_pool(name="ctxp", bufs=2) as ctxp, \
             tc.tile_pool(name="stream", bufs=4) as stream, \
             tc.tile_pool(name="small", bufs=2) as small, \
             tc.tile_pool(name="psA", bufs=2, space="PSUM") as psA, \
             tc.tile_pool(name="scr", bufs=3, space="PSUM") as scr, \
             tc.tile_pool(name="accp", bufs=2, space="PSUM") as accp:

            # ---- staged constants/weights; DMA order = priority order ----
            # (first projection chain is gated only on xt[d0:4] + wq[d0:4])
            def stage_xt(sub):
                xt = xtc.tile([128, ND, SUB], DT, tag="xt", name=f"xt{sub}")
                csl = slice(sub * SUB, (sub + 1) * SUB)
                for dpart in range(0, ND, 4):
                    dsl = slice(dpart, dpart + 4)
                    nc.sync.dma_start(out=xt[:, dsl, :], in_=xtp[:, dsl, csl])
                return xt

            wq_sb = stat.tile([128, ND, HL * HD], DT, tag="wq")
            wkvd_sb = stat.tile([128, ND, KVL], DT, tag="wkvd")
            xt0 = xtc.tile([128, ND, SUB], DT, tag="xt", name="xt0")
            for dpart in range(0, ND, 4):
                dsl = slice(dpart, dpart + 4)
                nc.sync.dma_start(out=xt0[:, dsl, :], in_=xtp[:, dsl, 0:SUB])
                nc.sync.dma_start(out=wkvd_sb[:, dsl, :],
                                  in_=wkvdp[:, dsl, :])
                nc.sync.dma_start(out=wq_sb[:, dsl, :], in_=wqp[:, dsl, :])
            wkc_sb = stat.tile([128, KVL // 128, HL * CD], DT, tag="wkc")
            nc.sync.dma_start(out=wkc_sb[:], in_=wkcp[:])
            wvu_sb = stat.tile([128, KVL // 128, HL * HD], DT, tag="wvu")
            nc.sync.dma_start(out=wvu_sb[:], in_=wvup[:])
            wkr_sb = stat.tile([128, ND, RD], DT, tag="wkr")
            nc.sync.dma_start(out=wkr_sb[:], in_=wkrp[:])
            cosT = stat.tile([128, S], DT, tag="cos")
            sinT = stat.tile([128, S], DT, tag="sin")
            nc.sync.dma_start(out=cosT[64:128, :], in_=cosd[:])
            nc.sync.dma_start(out=sinT[64:128, :], in_=sind[:])
            ident = stat.tile([128, 128], DT, tag="ident")
            nc.sync.dma_start(out=ident[:], in_=identd[:])
            p64 = stat.tile([128, RD], DT, tag="p64")
            nc.sync.dma_start(out=p64[64:128, :], in_=p64d[:])
            tri = stat.tile([128, 128], DT, tag="tri")
            nc.sync.dma_start(out=tri[:], in_=trid[:])
            ones = stat.tile([128, 128], DT, tag="ones")
            nc.sync.dma_start(out=ones[:], in_=onesd[:])
            wo_sb = stat.tile([128, HL, D], DT, tag="wo")

            qT = [seq.tile([128, S], DT, tag=f"qT{l}", name=f"qT{l}")
                  for l in range(HL)]
            kT = [seq.tile([128, S], DT, tag=f"kT{l}", name=f"kT{l}")
                  for l in range(HL)]
            vN = [seq.tile([128, S], DT, tag=f"vN{l}", name=f"vN{l}")
                  for l in range(HL)]

            # ---------------- projections ----------------
            # chain order per sub: q(0..3), k(0..3), v(0..3) — the first 4
            # chains are gated only on wq+xt (4MB); wk/wv stream in their
            # shadow. RoPE rotations ride along the k chains; v transposes
            # are deferred one chain so the vt drain never stalls PE.
            def chain(w_sb, l, xt):
                ps = psA.tile([128, SUB], F32, tag="mm", name="ps")
                for d in range(ND):
                    nc.tensor.matmul(ps[:], w_sb[:, d, l * 128:(l + 1) * 128],
                                     xt[:, d, :], start=d == 0,
                                     stop=d == ND - 1)
                return ps

            def emit_vtp(vt_sb, l, sub):
                # vT chunk -> natural layout via PE transpose
                for s4 in range(0, SUB // 128, 2):
                    tp = scr.tile([128, 256], DT, tag="scr", name="tp")
                    nc.tensor.transpose(
                        tp[:, 0:128],
                        vt_sb[:, s4 * 128:(s4 + 1) * 128], ident[:])
                    nc.tensor.transpose(
                        tp[:, 128:256],
                        vt_sb[:, (s4 + 1) * 128:(s4 + 2) * 128], ident[:])
                    col = sub * SUB + s4 * 128
                    if s4 == 0:
                        nc.scalar.copy(vN[l][:, col:col + 256], tp[:, 0:256])
                    else:
                        nc.vector.tensor_copy(vN[l][:, col:col + 256],
                                              tp[:, 0:256])

            # kv-latent: c_kv = x @ W_kv_down once (4 chains), then K=512
            # up-projections per head — 16 fewer K=2048 chains per sub.
            # c_kv for sub q+1 is emitted at the END of sub q so its
            # PSUM->SBUF drains complete during attention quad q instead
            # of stalling the next sub's kc/v chains.
            def emit_ckv(xt):
                ckv = []
                for j in range(KVL // 128):
                    c_ps = chain(wkvd_sb, j, xt)
                    ck = small.tile([128, SUB], DT, tag=f"ckv{j}",
                                    name=f"ckv{j}")
                    if j % 2 == 0:
                        nc.scalar.copy(ck[:], c_ps[:])
                    else:
                        nc.vector.tensor_copy(ck[:], c_ps[:])
                    ckv.append(ck)
                return ckv

            def emit_proj_sub(sub, xt, ckv):
                sl = slice(sub * SUB, (sub + 1) * SUB)
                for l in range(HL):
                    q_ps = chain(wq_sb, l, xt)
                    nc.vector.tensor_copy(qT[l][:, sl], q_ps[:])
                vts = []
                for l in range(HL):
                    k_ps = psA.tile([128, SUB], F32, tag="mm", name="k_ps")
                    for j in range(KVL // 128):
                        nc.tensor.matmul(
                            k_ps[0:CD, :],
                            wkc_sb[:, j, l * CD:(l + 1) * CD],
                            ckv[j][:], start=j == 0, stop=j == KVL // 128 - 1)
                    if l == 0:
                        # shared rope key rides in head-0's psum rows 64:128
                        for d in range(ND):
                            nc.tensor.matmul(
                                k_ps[64:128, :], wkr_sb[:, d, :],
                                xt[:, d, :], start=d == 0, stop=d == ND - 1)
                    # v up-projection rides with kc: together they give PE
                    # enough work per head to cover the DVE rope chain
                    v_ps = psA.tile([128, SUB], F32, tag="mm", name="v_ps")
                    for j in range(KVL // 128):
                        nc.tensor.matmul(
                            v_ps[:], wvu_sb[:, j, l * HD:(l + 1) * HD],
                            ckv[j][:], start=j == 0, stop=j == KVL // 128 - 1)
                    if l == 0:
                        nc.vector.tensor_copy(kT[0][:, sl], k_ps[:])
                    else:
                        nc.vector.tensor_copy(kT[l][0:CD, sl], k_ps[0:CD, :])
                    vt_sb = small.tile([128, SUB], DT, tag="vtsb", bufs=3)
                    nc.scalar.copy(vt_sb[:], v_ps[:])
                    vts.append(vt_sb)
                    # rope rotation matmuls ride between chains
                    rot = scr.tile([128, SUB], F32, tag="scr")
                    nc.tensor.matmul(rot[0:64, :], p64[64:128, :],
                                     qT[l][64:128, sl], start=True, stop=True)
                    tmp = small.tile([128, SUB], F32, tag="ropetmp")
                    nc.vector.tensor_tensor(qT[l][64:128, sl],
                                            qT[l][64:128, sl],
                                            cosT[64:128, sl], op=ALU.mult)
                    nc.vector.tensor_tensor(tmp[64:128, :], rot[0:64, :],
                                            sinT[64:128, sl], op=ALU.mult)
                    nc.vector.tensor_tensor(qT[l][64:128, sl],
                                            qT[l][64:128, sl],
                                            tmp[64:128, :], op=ALU.add)
                    if l == 0:
                        rotk = scr.tile([128, SUB], F32, tag="scr")
                        nc.tensor.matmul(rotk[0:64, :], p64[64:128, :],
                                         kT[0][64:128, sl],
                                         start=True, stop=True)
                        tmpk = small.tile([128, SUB], F32, tag="ropetmp")
                        nc.vector.tensor_tensor(kT[0][64:128, sl],
                                                kT[0][64:128, sl],
                                                cosT[64:128, sl], op=ALU.mult)
                        nc.vector.tensor_tensor(tmpk[64:128, :], rotk[0:64, :],
                                                sinT[64:128, sl], op=ALU.mult)
                        nc.vector.tensor_tensor(kT[0][64:128, sl],
                                                kT[0][64:128, sl],
                                                tmpk[64:128, :], op=ALU.add)
                    else:
                        # shared rope key: reuse head 0's rotated rows
                        nc.vector.tensor_copy(kT[l][64:128, sl],
                                              kT[0][64:128, sl])
                    if l > 0:
                        emit_vtp(vts[l - 1], l - 1, sub)
                emit_vtp(vts[HL - 1], HL - 1, sub)

            # -------- attention (quads of 4 q-blocks) + out-projection -----
            LOOK = 3
            ctx_hist = {}

            def outproj_qh(q, qh):
                row0 = (4 * q + qh) * 128
                for n in range(4):
                    op_ps = psA.tile([128, 512], F32, tag="mm")
                    for l in range(HL):
                        nc.tensor.matmul(
                            op_ps[:],
                            ctx_hist[q][l][:, qh * 128:(qh + 1) * 128],
                            wo_sb[:, l, n * 512:(n + 1) * 512],
                            start=l == 0, stop=l == HL - 1)
                    ob = stream.tile([128, 512], ODT, tag="ob")
                    if n % 2 == 0:
                        nc.scalar.copy(ob[:], op_ps[:])
                    else:
                        nc.vector.tensor_copy(ob[:], op_ps[:])
                    nc.sync.dma_start(
                        out=outd[row0:row0 + 128, n * 512:(n + 1) * 512],
                        in_=ob[:])

            # finalize (softmax denominator + normalized ctx copy) for head
            # (q, l) is DEFERRED into the next head's section, split in two:
            # part A (DVE merge + PE rowsum matmul) lands early where its
            # inputs are long ready; part B (ACT Ln/Exp + DVE mult) lands
            # after the section's scores so it sits in the ACT idle slot
            # instead of delaying the next head's Exp stream.
            pend_a = [None]
            pend_b = [None]
            pend_tail = [None]      # deferred AV tail of the previous head

            def fin_a():
                st = pend_a[0]
                if st is None:
                    return
                pend_a[0] = None
                fq, fl, acc, racg, racv = st
                rbf = small.tile([128, 512], DT, tag="rbf")
                nc.vector.tensor_tensor(rbf[:], racg[:], racv[:], op=ALU.add)
                rs = accp.tile([128, 512], F32, tag="rs", bufs=1, name="rs")
                nc.tensor.matmul(rs[:], ones[:], rbf[:], start=True,
                                 stop=True)
                # drain the rowsum bank right away so its single PSUM buf
                # never serializes the next head's rowsum matmul
                rs_sb = small.tile([128, 512], F32, tag="rssb")
                nc.vector.tensor_copy(rs_sb[:], rs[:])
                pend_b[0] = (fq, fl, acc, rs_sb)

            def fin_b():
                st = pend_b[0]
                if st is None:
                    return
                pend_b[0] = None
                fq, fl, acc, rs_sb = st
                lnr = small.tile([128, 512], F32, tag="lnr")
                nc.scalar.activation(lnr[:], rs_sb[:], AF.Ln)
                rc = small.tile([128, 512], F32, tag="rc")
                nc.scalar.activation(rc[:], lnr[:], AF.Exp, scale=-1.0)
                ct = ctxp.tile([128, 512], DT, tag=f"ct{fl}")
                nc.vector.tensor_tensor(ct[:], acc[:], rc[:], op=ALU.mult)
                ctx_hist[fq].append(ct)

            def emit_attn_quad(q):
                q0 = 4 * q                      # first q-block of the quad
                nbk = 4 * q + 4                 # k blocks 0..nbk-1
                ctx_hist[q] = []
                for l in range(HL):
                    acc = accp.tile([128, 512], F32, tag="acc")
                    # exp-sum side chains, split even/odd blocks across the
                    # GpSimd and Vector engines (GpSimd alone can't keep pace)
                    racg = small.tile([128, 512], F32, tag="racg")
                    racv = small.tile([128, 512], F32, tag="racv",
                                      name="racv")
                    at_tiles = [None] * nbk

                    def emit_score(kb):
                        i0 = max(kb - q0, 0)    # first valid q-block index
                        W = 512 - 128 * i0
                        c0 = 512 - W
                        sT = scr.tile([128, 512], F32, tag="scr")
                        nc.tensor.matmul(
                            sT[:, 0:W],
                            kT[l][:, kb * 128:(kb + 1) * 128],
                            qT[l][:, q0 * 128 + c0: (q0 + 4) * 128],
                            start=True, stop=True)
                        at = atp.tile([128, 512], DT, tag="at")
                        nc.scalar.activation(at[:, 0:W], sT[:, 0:W], AF.Exp)
                        if kb >= q0:
                            # diagonal q-block sits at tile cols 0:128
                            nc.vector.tensor_tensor(
                                at[:, 0:128], at[:, 0:128], tri[:],
                                op=ALU.mult)
                        eng, r = (nc.gpsimd, racg) if kb % 2 == 0 \
                            else (nc.vector, racv)
                        if kb < 2:
                            # memset + add beats tensor_copy: the bf16->f32
                            # cast copy runs ~1.9us vs memset 0.14 + add 0.7
                            eng.memset(r[:], 0.0)
                        eng.tensor_tensor(r[:, c0:512], r[:, c0:512],
                                          at[:, 0:W], op=ALU.add)
                        at_tiles[kb] = at

                    def emit_av(kb, acc=acc, at_tiles=at_tiles, l=l, q0=q0):
                        i0 = max(kb - q0, 0)
                        W = 512 - 128 * i0
                        c0 = 512 - W
                        at = at_tiles[kb]
                        vblk = vN[l][:, kb * 128:(kb + 1) * 128]
                        if kb < q0:
                            nc.tensor.matmul(acc[:, 0:512], vblk, at[:, 0:512],
                                             start=kb == 0, stop=False)
                        else:
                            # diag block: cols 0:128 of the tile finish
                            # q-block i0; the rest continue accumulating
                            nc.tensor.matmul(
                                acc[:, c0:c0 + 128], vblk, at[:, 0:128],
                                start=kb == 0, stop=True)
                            if W > 128:
                                nc.tensor.matmul(
                                    acc[:, c0 + 128:512], vblk, at[:, 128:W],
                                    start=kb == 0, stop=i0 == 3)

                    tail = pend_tail[0]
                    pend_tail[0] = None
                    for kb in range(nbk):
                        emit_score(kb)
                        # the previous head's AV tail rides in the first
                        # score slots, where its exps are long done
                        if tail and kb < len(tail):
                            tail[kb]()
                        if kb == 1:
                            fin_a()
                        if kb >= LOOK:
                            emit_av(kb - LOOK)
                    pend_tail[0] = [functools.partial(emit_av, kb)
                                    for kb in range(nbk - LOOK, nbk)]

                    fin_b()
                    pend_a[0] = (q, l, acc, racg, racv)
                    # interleave one q-block of the previous quad's
                    # out-projection per head: PE burst with no ACT deps,
                    # so the Exp pipeline gets a catch-up window
                    if q > 0:
                        outproj_qh(q - 1, l)

            # ---- driver: projection sub q feeds attention quad q; the
            # interleave mixes the PE-heavy/ACT-light projection stream
            # with the ACT-heavy attention stream so Exp never falls
            # behind, and spreads DMA in/out across the whole kernel ----
            xt_cur = xt0
            ckv_next = None
            for q in range(NQUAD):
                xt = xt_cur
                if q + 1 < NSUB:
                    xt_cur = stage_xt(q + 1)
                if q == 1:
                    # wo is first needed at the out-projection (quad 1);
                    # load it in the shadow of sub-1 projection compute
                    for lh in range(0, HL, 2):
                        nc.sync.dma_start(out=wo_sb[:, lh:lh + 2, :],
                                          in_=wop[:, lh:lh + 2, :])
                ckv_q = emit_ckv(xt) if q == 0 else ckv_next
                emit_proj_sub(q, xt, ckv_q)
                if q + 1 < NQUAD:
                    ckv_next = emit_ckv(xt_cur)
                emit_attn_quad(q)
            for f in pend_tail[0] or []:
                f()
            fin_a()
            fin_b()
            for qh in range(4):
                outproj_qh(NQUAD - 1, qh)

    return nc


# ---------------------------------------------------------------------------
# Host side
# ---------------------------------------------------------------------------
def _rope_tables():
    inv_freq = 1.0 / (10000.0 ** (np.arange(0, RD, 2, dtype=np.float32) / RD))
    t = np.arange(S, dtype=np.float32)
    freqs = np.outer(t, inv_freq).astype(np.float32)
    emb = np.concatenate([freqs, freqs], axis=-1)
    cos = np.cos(emb).astype(np.float32)    # [S, RD]
    sin = np.sin(emb).astype(np.float32)
    return np.ascontiguousarray(cos.T), np.ascontiguousarray(sin.T)


def _host_prep(x, W_kv_down, W_q_down, W_kc, W_v, W_qc, W_kr, W_qr, W_o, b_o):
    f = np.float32
    Wqc_f = (W_q_down @ W_qc).astype(f)       # [D, CD*H]
    Wqr_f = (W_q_down @ W_qr).astype(f)       # [D, RD*H]
    scale = f(1.0 / np.sqrt(np.float32(HD)))

    cosT, sinT = _rope_tables()

    ident = np.eye(128, dtype=f)
    p64 = np.zeros((RD, RD), f)
    for m in range(RD):
        if m < 32:
            p64[m + 32, m] = -1.0
        else:
            p64[m - 32, m] = 1.0
    tri01 = (np.arange(128)[:, None] <= np.arange(128)[None, :]).astype(f)
    ones128 = np.ones((128, 128), f)

    # packed per-group weights
    wq_g, wkc_g, wvu_g, wo_g = [], [], [], []
    for g in range(NG):
        wq_c = np.empty((D, HL * HD), f)
        wkc_c = np.empty((KVL, HL * CD), f)
        wvu_c = np.empty((KVL, HL * HD), f)
        wo_c = np.empty((HL * HD, D), f)
        for l in range(HL):
            h = HL * g + l
            wq_c[:, l * 128:l * 128 + 64] = \
                Wqc_f[:, h * 64:(h + 1) * 64] * scale
            wq_c[:, l * 128 + 64:(l + 1) * 128] = \
                Wqr_f[:, h * 64:(h + 1) * 64] * scale
            wkc_c[:, l * CD:(l + 1) * CD] = W_kc[:, h * CD:(h + 1) * CD]
            wvu_c[:, l * HD:(l + 1) * HD] = W_v[:, h * HD:(h + 1) * HD]
            wo_c[l * 128:(l + 1) * 128, :] = W_o[h * 128:(h + 1) * 128, :]
        # [D, C] -> [128, ND, C];  [KVL, C] -> [128, KVL//128, C]
        wq_g.append(np.ascontiguousarray(
            wq_c.reshape(ND, 128, HL * HD).transpose(1, 0, 2)))
        wkc_g.append(np.ascontiguousarray(
            wkc_c.reshape(KVL // 128, 128, HL * CD).transpose(1, 0, 2)))
        wvu_g.append(np.ascontiguousarray(
            wvu_c.reshape(KVL // 128, 128, HL * HD).transpose(1, 0, 2)))
        # [HL*128, D] -> [128, HL, D]
        wo_g.append(np.ascontiguousarray(
            wo_c.reshape(HL, 128, D).transpose(1, 0, 2)))
    wkvd_p = np.ascontiguousarray(
        W_kv_down.astype(f).reshape(ND, 128, KVL).transpose(1, 0, 2))
    wkr_p = np.ascontiguousarray(
        W_kr.astype(f).reshape(ND, 128, RD).transpose(1, 0, 2))

    # packed x per batch: [128, ND, S];  xtp[p, d, t] = x[b, t, d*128+p]
    xtp_b = []
    for b in range(NB):
        xb = x[b]                              # [S, D]
        xtp_b.append(np.ascontiguousarray(
            xb.T.reshape(ND, 128, S).transpose(1, 0, 2)))

    in_maps = []
    for c in range(NC):
        b, g = c // NG, c % NG
        in_maps.append({
            "xtp": xtp_b[b], "wqp": wq_g[g], "wkvdp": wkvd_p,
            "wkcp": wkc_g[g], "wkrp": wkr_p, "wvup": wvu_g[g],
            "wop": wo_g[g],
            "cosd": cosT, "sind": sinT,
            "identd": ident, "p64d": p64, "trid": tri01, "onesd": ones128,
        })
    if _DT_NAME == "bf16":
        import ml_dtypes
        bf = ml_dtypes.bfloat16
        in_maps = [{k: v.astype(bf) for k, v in m.items()} for m in in_maps]
    return in_maps


def kernel(**inputs):
    inputs = {k: np.asarray(v, np.float32) for k, v in inputs.items()}
    if "nc" not in _CACHE:
        prog = _build_program()
        _split_multi_waits(prog)
        _CACHE["nc"] = prog
    prog = _CACHE["nc"]
    in_maps = _host_prep(**inputs)
    res = None
    for attempt in range(3):
        try:
            res = run_bass_kernel_spmd(prog, in_maps, core_ids=list(range(NC)))
            break
        except Exception:
            if attempt == 2:
                raise
            import time
            time.sleep(5.0)
    out = np.zeros((B, S, D), np.float32)
    for c, r in enumerate(res.results):
        out[c // NG] += np.asarray(r["out"], np.float32)
    out += inputs["b_o"][None, None, :]
    return out
